# revision 3
# baseline (speedup 1.0000x reference)
"""Multi-head attention TRN2 Bass kernel (8 NeuronCores, tensor-parallel).

Sharding: Megatron-style TP over (batch x head-group). 8 cores = 2 batches x 4
head-groups of 4 heads each. Each core computes its heads' Q/K/V projections,
masked-softmax attention, and per-head-pair partial output projections; the
host sums the 8 partials per batch (the TP unshard).

v2 schedule: the ACT engine's exp stream (~147us of work) is the critical
resource, so everything is arranged to keep it saturated from ~t=30us:
  - k-proj then q-proj run first (DMA-gated), scores for head 0 start
    immediately after; v-proj is interleaved into head 0's PE slack
  - mask multiplies are split between DVE and GPSIMD
  - all PSUM->SBUF drains go to DVE (ACT does exp only during attention)
  - out-projection is split per head-pair (K=128 contraction each) so it
    overlaps attention; the host sums 2 partial outputs per core
"""
import os
import sys

for p in ("/opt/trn_rl_repo",):
    if p not in sys.path:
        sys.path.insert(0, p)

from contextlib import ExitStack

import numpy as np

import concourse.bass as bass
import concourse.tile as tile
from concourse import bacc, mybir
from concourse.bass_utils import run_bass_kernel_spmd

F32 = mybir.dt.float32
F16 = mybir.dt.float16
EXP = mybir.ActivationFunctionType.Exp

B, M, N, E = 2, 2048, 2048, 1024  # batch, q-len, k-len, d_model
H, DK = 16, 64                    # heads, head dim
NCORES = 8
GROUPS = 4                        # head groups (cores per batch)
DLOC = (H // GROUPS) * DK         # 256 per-core projection width
HL = H // GROUPS                  # 4 local heads
ET = E // 128                     # 8 e-tiles
NT = N // 128                     # 16 n-tiles
VSTR = HL * (DK + 1)              # 260: vw slot stride per n-tile

DEPTH = int(os.environ.get("K_DEPTH", "2"))
S_BUFS = int(os.environ.get("K_SBUFS", "3"))
AU_BUFS = int(os.environ.get("K_AUBUFS", "3"))
AM_BUFS = int(os.environ.get("K_AMBUFS", "18"))
GP_EVERY = int(os.environ.get("K_GPEVERY", "2"))  # every GP_EVERY-th mask tile on gpsimd


def build_program() -> bass.Bass:
    nc = bacc.Bacc()

    qT_d = nc.dram_tensor("qT", [E, M], F16, kind="ExternalInput")
    kT_d = nc.dram_tensor("kT", [E, N], F16, kind="ExternalInput")
    vT_d = nc.dram_tensor("vT", [E, N], F16, kind="ExternalInput")
    keepT_d = nc.dram_tensor("keepT", [N, M], F16, kind="ExternalInput")
    wqT_d = nc.dram_tensor("wqT", [E, DLOC], F16, kind="ExternalInput")
    wkT_d = nc.dram_tensor("wkT", [E, DLOC], F16, kind="ExternalInput")
    wvT_d = nc.dram_tensor("wvT", [E, DLOC], F16, kind="ExternalInput")
    woT_d = nc.dram_tensor("woT", [DLOC, E], F16, kind="ExternalInput")
    out0_d = nc.dram_tensor("out0", [M, E], F16, kind="ExternalOutput")
    out1_d = nc.dram_tensor("out1", [M, E], F16, kind="ExternalOutput")
    out_ds = [out0_d, out1_d]

    with tile.TileContext(nc) as tc, ExitStack() as ctx:
        const_pool = ctx.enter_context(tc.tile_pool(name="const", bufs=1))
        w_pool = ctx.enter_context(tc.tile_pool(name="weights", bufs=1))
        act_pool = ctx.enter_context(tc.tile_pool(name="acts", bufs=1))
        keep_pool = ctx.enter_context(tc.tile_pool(name="keep", bufs=1))

        ones64 = const_pool.tile([1, 64], F16)
        nc.vector.memset(ones64[:], 1.0)
        warm_exp = const_pool.tile([1, 64], F16)
        nc.scalar.activation(warm_exp[:], ones64[:], EXP, scale=0.125)

        wq_sb = w_pool.tile([128, ET * DLOC], F16, tag="wq")
        wk_sb = w_pool.tile([128, ET * DLOC], F16, tag="wk")
        wv_sb = w_pool.tile([128, ET * DLOC], F16, tag="wv")
        wo_sb = w_pool.tile([128, 2 * E], F16, tag="wo")

        # qwz[hp][hl]: full-128-partition qw with the OTHER head's 64 rows
        # zeroed -> scores matmuls use K=128 with a single shared kw lhsT.
        qwz = [
            [act_pool.tile([128, M], F16, tag=f"qwz{i}{j}", name=f"qwz{i}{j}")
             for j in range(2)]
            for i in range(2)
        ]
        for i in range(2):
            nc.vector.memset(qwz[i][0][bass.ts(1, 64), :], 0.0)
            nc.vector.memset(qwz[i][1][bass.ts(0, 64), :], 0.0)
        kw_sb = [act_pool.tile([128, N], F16, tag=f"kw{i}", name=f"kw{i}")
                 for i in range(2)]
        vw_sb = act_pool.tile([128, NT * VSTR], F16, tag="vw")
        ctx_sb = [act_pool.tile([128, M], F16, tag=f"ctx{i}", name=f"ctx{i}")
                  for i in range(2)]
        nc.vector.memset(vw_sb[:], 1.0)  # ones cols; data cols overwritten

        # full mask resident: (n-tile, m) per nt slot of 2048
        keep_sb = keep_pool.tile([128, NT * M], F16, tag="keep")

        with tc.tile_pool(name="vT", bufs=8) as vT_pool:
            vts = [vT_pool.tile([128, N], F16, tag="vt", name=f"vt{et}")
                   for et in range(ET)]

            # ---- input DMA issue order == priority order ----
            with tc.tile_pool(name="xT", bufs=3) as xT_pool:
                nc.sync.dma_start(
                    wk_sb.rearrange("p (t d) -> p t d", t=ET),
                    wkT_d.rearrange("(t p) d -> p t d", p=128),
                )
                kts = []
                for et in range(ET):
                    kt = xT_pool.tile([128, N], F16, tag="xt", name=f"kt{et}")
                    nc.sync.dma_start(kt[:], kT_d[bass.ts(et, 128), :])
                    kts.append(kt)
                nc.sync.dma_start(
                    wq_sb.rearrange("p (t d) -> p t d", t=ET),
                    wqT_d.rearrange("(t p) d -> p t d", p=128),
                )
                qts = []
                for et in range(ET):
                    qt = xT_pool.tile([128, M], F16, tag="xt", name=f"qt{et}")
                    nc.sync.dma_start(qt[:], qT_d[bass.ts(et, 128), :])
                    qts.append(qt)
                # first mask tiles early: mask(h0, nt) is consumed from ~t=30us
                for ntc in range(4):
                    nc.sync.dma_start(
                        keep_sb[:, bass.ts(ntc, M)], keepT_d[bass.ts(ntc, 128), :]
                    )
                nc.sync.dma_start(
                    wv_sb.rearrange("p (t d) -> p t d", t=ET),
                    wvT_d.rearrange("(t p) d -> p t d", p=128),
                )
                for et in range(ET):
                    nc.sync.dma_start(vts[et][:], vT_d[bass.ts(et, 128), :])
                for ntc in range(4, NT):
                    nc.sync.dma_start(
                        keep_sb[:, bass.ts(ntc, M)], keepT_d[bass.ts(ntc, 128), :]
                    )
                nc.sync.dma_start(
                    wo_sb.rearrange("p (t d) -> p t d", t=2),
                    woT_d.rearrange("(t p) d -> p t d", p=128),
                )

                # ---- k then q projections (PSUM pool A: 8 banks) ----
                with tc.tile_pool(name="proj_ps", bufs=8, space="PSUM") as pps:
                    def proj_qk(xts, w_sb, writer):
                        ps = [pps.tile([128, 512], F32, tag="pp", name=f"pp{j2}")
                              for j2 in range(8)]
                        for et in range(ET):
                            for d2 in range(2):
                                for mc in range(4):
                                    nc.tensor.matmul(
                                        ps[d2 * 4 + mc][:],
                                        w_sb[:, et * DLOC + d2 * 128:
                                             et * DLOC + (d2 + 1) * 128],
                                        xts[et][:, bass.ts(mc, 512)],
                                        start=(et == 0), stop=(et == ET - 1),
                                    )
                        # drain order: d2=0 first (scores h0/h1 need it first)
                        for d2 in range(2):
                            for mc in range(4):
                                writer(d2, mc, ps[d2 * 4 + mc])

                    def k_writer(d2, mc, ps):
                        nc.vector.tensor_copy(kw_sb[d2][:, bass.ts(mc, 512)], ps[:])

                    def q_writer(d2, mc, ps):
                        nc.vector.tensor_copy(
                            qwz[d2][0][bass.ts(0, 64), bass.ts(mc, 512)],
                            ps[bass.ts(0, 64), :],
                        )
                        nc.vector.tensor_copy(
                            qwz[d2][1][bass.ts(1, 64), bass.ts(mc, 512)],
                            ps[bass.ts(1, 64), :],
                        )

                    proj_qk(kts, wk_sb, k_writer)
                    proj_qk(qts, wq_sb, q_writer)
            # xT pool released here (SBUF reclaimed before attn pools open)

            # ---- attention + v-proj + out-proj, one big pipeline ----
            with (
                tc.tile_pool(name="s_ps", bufs=S_BUFS, space="PSUM") as s_ps,
                tc.tile_pool(name="attn", bufs=AU_BUFS) as attn_pool,
                tc.tile_pool(name="attnm", bufs=AM_BUFS) as attnm_pool,
                tc.tile_pool(name="eps", bufs=1) as eps_pool,
                tc.tile_pool(name="ob", bufs=2) as ob_pool,
            ):
                vp_ctx = ExitStack()
                vp_ps = vp_ctx.enter_context(
                    tc.tile_pool(name="vp_ps", bufs=2, space="PSUM")
                )
                c_ctx = ExitStack()
                c_ps = None  # entered after v-proj pool exits (PSUM banks)

                def vproj_step(nt):
                    # one n-tile of the v projection: accumulate 8 e-tiles
                    vp = vp_ps.tile([128, 512], F32, tag="vp", name=f"vp{nt}")
                    for et in range(ET):
                        nc.tensor.matmul(
                            vp[:, 0:DLOC],
                            vts[et][:, bass.ts(nt, 128)],
                            wv_sb[:, bass.ts(et, DLOC)],
                            start=(et == 0), stop=(et == ET - 1),
                        )
                    # strided drain into the 65-wide head slots (ones col kept)
                    dst = vw_sb[:, nt * VSTR: nt * VSTR + VSTR].rearrange(
                        "p (h d) -> p h d", h=HL
                    )[:, :, 0:DK]
                    src = vp[:, 0:DLOC].rearrange("p (h d) -> p h d", h=HL)
                    nc.vector.tensor_copy(dst, src)

                for mh in range(2):  # m-halves of 1024
                    moff = mh * 1024
                    for h in range(HL):
                        hp, hl = divmod(h, 2)
                        first = (mh == 0 and h == 0)
                        # head 0's ctx waits for v-proj to release PSUM banks
                        d_eff = NT if first else DEPTH
                        pctx = None
                        ams = {}
                        for step in range(NT + d_eff):
                            if step < NT:
                                nt = step
                                ps = s_ps.tile([128, 1024], F32, tag="ps")
                                for mc2 in range(2):
                                    nc.tensor.matmul(
                                        ps[:, bass.ts(mc2, 512)],
                                        kw_sb[hp][:, bass.ts(nt, 128)],
                                        qwz[hp][hl][
                                            :,
                                            moff + mc2 * 512:
                                            moff + (mc2 + 1) * 512,
                                        ],
                                        start=True, stop=True,
                                    )
                                if first:
                                    vproj_step(nt)
                                au = attn_pool.tile([128, 1024], F16, tag="au")
                                nc.scalar.activation(au[:], ps[:], EXP, scale=0.125)
                                am = attnm_pool.tile([128, 1024], F16, tag="am")
                                keep_sl = keep_sb[
                                    :, nt * M + moff: nt * M + moff + 1024
                                ]
                                if GP_EVERY and nt % GP_EVERY == (GP_EVERY - 1):
                                    nc.gpsimd.tensor_mul(am[:], au[:], keep_sl)
                                else:
                                    nc.vector.tensor_mul(am[:], au[:], keep_sl)
                                ams[nt] = am
                            if step >= d_eff:
                                nt = step - d_eff
                                if nt == 0:
                                    if first:
                                        # v-proj banks -> ctx banks
                                        vp_ctx.close()
                                        c_ps = c_ctx.enter_context(
                                            tc.tile_pool(
                                                name="c_ps", bufs=1, space="PSUM"
                                            )
                                        )
                                    pctx = c_ps.tile(
                                        [65, 1024], F32, tag="pctx", name="pctx"
                                    )
                                am = ams.pop(nt)
                                for mc2 in range(2):
                                    nc.tensor.matmul(
                                        pctx[:, bass.ts(mc2, 512)],
                                        vw_sb[:, nt * VSTR + h * 65:
                                              nt * VSTR + (h + 1) * 65],
                                        am[:, bass.ts(mc2, 512)],
                                        start=(nt == 0), stop=(nt == NT - 1),
                                    )
                        # normalize: ctx = pctx[0:64] / sums ; sums = row 64
                        sums = eps_pool.tile([1, 1024], F16, tag="sums")
                        nc.vector.tensor_copy(sums[:], pctx[64:65, :])
                        prb = s_ps.tile([128, 1024], F32, tag="ps", name="prb")
                        for mc2 in range(2):
                            nc.tensor.matmul(
                                prb[0:64, bass.ts(mc2, 512)],
                                ones64[:],
                                sums[:, bass.ts(mc2, 512)],
                                start=True, stop=True,
                            )
                        rbs = eps_pool.tile([64, 1024], F32, tag="rbs")
                        nc.vector.reciprocal_approx_fast(rbs[:], prb[0:64, :])
                        nc.vector.tensor_mul(
                            ctx_sb[hp][bass.ts(hl, 64), moff: moff + 1024],
                            pctx[0:64, :],
                            rbs[:],
                        )
                        # out-proj for this (mh, head-pair) once both heads done
                        if hl == 1:
                            for mt in range(8):
                                po = c_ps.tile(
                                    [128, 1024], F32, tag="pctx", name=f"po{mt}"
                                )
                                for ec in range(2):
                                    nc.tensor.matmul(
                                        po[:, bass.ts(ec, 512)],
                                        ctx_sb[hp][:, moff + mt * 128:
                                                   moff + (mt + 1) * 128],
                                        wo_sb[:, hp * E + ec * 512:
                                              hp * E + (ec + 1) * 512],
                                        start=True, stop=True,
                                    )
                                ob = ob_pool.tile([128, 1024], F16, tag="ob")
                                nc.vector.tensor_copy(ob[:], po[:])
                                nc.sync.dma_start(
                                    out_ds[hp][moff + mt * 128:
                                               moff + (mt + 1) * 128, :],
                                    ob[:],
                                )
                c_ctx.close()

    nc.finalize()
    return nc


_PROGRAM = None


def _get_program():
    global _PROGRAM
    if _PROGRAM is None:
        _PROGRAM = build_program()
    return _PROGRAM


def _make_in_maps(q, k, v, mask, Wq, Wk, Wv, Wo):
    q = np.asarray(q, dtype=np.float32)
    k = np.asarray(k, dtype=np.float32)
    v = np.asarray(v, dtype=np.float32)
    mask = np.asarray(mask)
    Wq = np.asarray(Wq, dtype=np.float32)
    Wk = np.asarray(Wk, dtype=np.float32)
    Wv = np.asarray(Wv, dtype=np.float32)
    Wo = np.asarray(Wo, dtype=np.float32)

    per_batch = {}
    for b in range(B):
        per_batch[b] = dict(
            qT=np.ascontiguousarray(q[b].T.astype(np.float16)),
            kT=np.ascontiguousarray(k[b].T.astype(np.float16)),
            vT=np.ascontiguousarray(v[b].T.astype(np.float16)),
            keepT=np.ascontiguousarray(
                np.logical_not(mask[b]).T.astype(np.float16)
            ),
        )

    in_maps = []
    for c in range(NCORES):
        b, hg = divmod(c, GROUPS)
        sl = slice(hg * DLOC, (hg + 1) * DLOC)
        in_maps.append(
            dict(
                per_batch[b],
                wqT=np.ascontiguousarray(Wq[sl].T.astype(np.float16)),
                wkT=np.ascontiguousarray(Wk[sl].T.astype(np.float16)),
                wvT=np.ascontiguousarray(Wv[sl].T.astype(np.float16)),
                woT=np.ascontiguousarray(Wo[:, sl].T.astype(np.float16)),
            )
        )
    return in_maps


def _run(in_maps, trace=False):
    nc = _get_program()
    return run_bass_kernel_spmd(
        nc, in_maps, list(range(NCORES)), trace=trace
    )


def _assemble(results):
    out = np.zeros((B, M, E), dtype=np.float32)
    for c in range(NCORES):
        b = c // GROUPS
        out[b] += results[c]["out0"].astype(np.float32)
        out[b] += results[c]["out1"].astype(np.float32)
    return out


def kernel(q, k, v, mask, Wq, Wk, Wv, Wo):
    in_maps = _make_in_maps(q, k, v, mask, Wq, Wk, Wv, Wo)
    res = _run(in_maps, trace=False)
    return _assemble(res.results)


def run_profiled(q, k, v, mask, Wq, Wk, Wv, Wo):
    """Like kernel(), but traces execution; returns (out, BassKernelResults)."""
    in_maps = _make_in_maps(q, k, v, mask, Wq, Wk, Wv, Wo)
    res = _run(in_maps, trace=True)
    return _assemble(res.results), res
